# revision 2
# baseline (speedup 1.0000x reference)
"""CRF loss (forward-algorithm denominator + gold-path numerator) on 8 trn2 cores.

Linear-space chain-parallel forward (see v1 docstring), with:
- int8 emissions (q = round(24*logit)); ACT applies exp(q/24 - CLVL) via scale+bias.
- W=1 warmup (contraction ~0.01/step; host-validated rel err ~3e-5).
- NO arena dump: on-device reductions r[s,p] = expend^T @ v_state accumulate into
  distinct PSUM rows (indicator-column lhsT strip); the two final-phase junction
  states ship raw (host reduces those 2x[T,512] itself) so the output tail is one
  small DMA chain, with the reduction rows leaving one phase earlier, hidden.
- fill tuned: em part 0 then one combined param tensor (E|init0|evstrip, all
  host-precomputed incl. exp) lead the DMA queue; em parts and exp blocks sized
  so the exp pipeline stays ahead of the DVE multiply stream.
- arena is phase-major so the two final states are adjacent (single vlast DMA).
"""

import ml_dtypes
import numpy as np

B, L, T = 256, 512, 128
NCORES = 8
BL = B // NCORES  # 32
G = 32            # parallel time chains
W = 1             # warmup steps per chain
WIN = L // G      # 16
NPH = W + WIN     # 17 phases (1..NPH), states 0..NPH
NST = NPH + 1     # 18 states stored per stream
CPS = G // 2      # chains per supertile (stream)
SCOL = CPS * BL   # stream width: 512 cols
PCOL = G * BL     # per-phase emission columns: 1024
QS = 24.0         # int8 emission quantization scale
CLVL = float(np.log(128.0) + 0.5)

# emission DMA parts in phases [start, end) 0-based; ACT-exp blocks in
# half-phase (SCOL-column) units, sized so exp supply stays ahead of the
# 1.32us/phase DVE consumption while the first multiply starts early
DMA_PARTS = [(0, 1), (1, 2), (2, 3), (3, 5), (5, 8), (8, 12), (12, NPH)]
EXP_BLOCKS = [(0, 1), (1, 2), (2, 3), (3, 4), (4, 6), (6, 8), (8, 10),
              (10, 14), (14, 18), (18, 22), (22, 28), (28, 2 * NPH)]
assert EXP_BLOCKS[-1][1] == 2 * NPH and DMA_PARTS[-1][1] == NPH
BLOCK_PART = [next(pi for pi, (da, db) in enumerate(DMA_PARTS)
                   if (b1 + 1) // 2 <= db)
              for (_, b1) in EXP_BLOCKS]
NB = len(EXP_BLOCKS)

bf16 = ml_dtypes.bfloat16


def _t_of(g: int, p: int) -> int:
    """time index of chain g's state p (may exceed L-1; caller clamps)."""
    return p if g == 0 else WIN * g - W + p


def _endpoint_state(t: int):
    """(g, p) of the canonical state holding alpha_t."""
    if t < WIN:
        return 0, t
    g = min(t // WIN, G - 1)
    return g, t - (WIN * g - W)


def _red_states(lengths):
    """Ordered list of (s, p) supertile states to reduce on device.
    The (s, NPH) junction anchors are excluded: raw final states ship instead,
    so every reduced state has p <= NPH-1."""
    need = set()
    for g in range(1, G):
        if g == 1:
            need.add((0, WIN))
        need.add((g // CPS, W))
    for ln in lengths:
        g, p = _endpoint_state(int(ln) - 1)
        need.add((g // CPS, p))
    # k-order must match the PE program's emission order: by (p, s)
    return sorted(need, key=lambda sp: (sp[1], sp[0]))


def _build_nc(red_states):
    import concourse.bass as bass
    import concourse.mybir as mybir
    from contextlib import ExitStack

    f32 = mybir.dt.float32
    b16 = mybir.dt.bfloat16
    i8 = mybir.dt.int8
    Exp = mybir.ActivationFunctionType.Exp
    Copy = mybir.ActivationFunctionType.Copy
    mult = mybir.AluOpType.mult

    NRED = len(red_states)
    assert NRED <= 32
    assert all(p <= NPH - 1 for _, p in red_states)
    red_idx = {sp: k for k, sp in enumerate(red_states)}

    blk_of_hu = {}
    for bi, (b0, b1) in enumerate(EXP_BLOCKS):
        for hu in range(b0, b1):
            blk_of_hu[hu] = bi

    nc = bass.Bass()
    emq_d = nc.dram_tensor("emq", [T, NPH * PCOL], i8, kind="ExternalInput").ap()
    # params: exp(trans) [0:128] | exp(start+logit0) [128:160] | evstrip
    # [160:223] | pad to 512B/partition so the DMA runs full-speed
    params_d = nc.dram_tensor("params", [T, 256], b16, kind="ExternalInput").ap()
    red_d = nc.dram_tensor("red", [32, SCOL], b16, kind="ExternalOutput").ap()
    vlast_d = nc.dram_tensor("vlast", [T, 2 * SCOL], b16, kind="ExternalOutput").ap()

    st = ExitStack()
    with st:
        params_sb = st.enter_context(nc.sbuf_tensor("params_sb", [T, 256], b16))
        bias_sb = st.enter_context(nc.sbuf_tensor("bias_sb", [T, 1], f32))
        emq_sb = st.enter_context(nc.sbuf_tensor("emq_sb", [T, NPH * PCOL], i8))
        ex_sb = st.enter_context(nc.sbuf_tensor("ex_sb", [T, NPH * PCOL], b16))
        arena = st.enter_context(nc.sbuf_tensor("arena", [T, 2 * NST * SCOL], b16))
        red_sb = st.enter_context(nc.sbuf_tensor("red_sb", [32, SCOL], b16))
        ps0 = st.enter_context(nc.psum_tensor("ps0", [T, SCOL], f32))
        ps1 = st.enter_context(nc.psum_tensor("ps1", [T, SCOL], f32))
        ps2 = st.enter_context(nc.psum_tensor("ps2", [T, SCOL], f32))
        ps3 = st.enter_context(nc.psum_tensor("ps3", [T, SCOL], f32))
        psr = st.enter_context(nc.psum_tensor("psr", [32, SCOL], f32))
        psd = st.enter_context(nc.psum_tensor("psd", [T, SCOL], f32))
        dma_in = st.enter_context(nc.semaphore("dma_in"))
        em_sem = st.enter_context(nc.semaphore("em_sem"))
        act_sem = st.enter_context(nc.semaphore("act_sem"))
        dve_sem = st.enter_context(nc.semaphore("dve_sem"))
        pe_sem = st.enter_context(nc.semaphore("pe_sem"))
        out_sem = st.enter_context(nc.semaphore("out_sem"))
        block = st.enter_context(nc.Block())

        psb = [ps0, ps1, ps2, ps3]
        E_ap = params_sb[:, 0:128]
        init_ap = params_sb[:, 128:160]
        evstrip = params_sb[:, 160:223]

        def astate(s, p):
            # phase-major: the two final states end up adjacent
            return arena[:, (2 * p + s) * SCOL:(2 * p + s + 1) * SCOL]

        # dve_sem after TT(p,s): bias, 2 memsets, init copy = 4, then 2/phase
        def tt_done(p, s):
            return 5 + 2 * (p - 1) + s

        # pe_sem bookkeeping (must mirror the tensor block's emission order)
        mm_index = {}
        ctr = 0
        last_red_count = 0
        for s in range(2):
            if (s, 0) in red_idx:
                ctr += 1
                last_red_count = ctr
        for p in range(1, NPH + 1):
            for s in range(2):
                if p == NPH and s == 1 and (s, p - 1) in red_idx:
                    ctr += 1          # the last reduction goes first so the
                    last_red_count = ctr  # psum eviction can start early
                ctr += 1
                mm_index[(p, s)] = ctr
                if (s, p - 1) in red_idx and p >= 2 and not (p == NPH and s == 1):
                    ctr += 1
                    last_red_count = max(last_red_count, ctr)
        n_mms = ctr
        assert n_mms == 2 * NPH + NRED

        @block.sync
        def _(sync):
            # em part 0 feeds the first exp block; params feed the first MM
            d0, d1 = DMA_PARTS[0]
            sync.dma_start(emq_sb[:, d0 * PCOL:d1 * PCOL],
                           emq_d[:, d0 * PCOL:d1 * PCOL]).then_inc(em_sem, 16)
            sync.dma_start(params_sb[:], params_d[:]).then_inc(dma_in, 16)
            for d0, d1 in DMA_PARTS[1:]:
                lo, hi = d0 * PCOL, d1 * PCOL
                sync.dma_start(emq_sb[:, lo:hi], emq_d[:, lo:hi]).then_inc(em_sem, 16)
            # the two adjacent final states leave first (critical tail);
            # the reduction rows were evicted earlier and follow behind
            sync.wait_ge(dve_sem, tt_done(NPH, 1))
            sync.dma_start(
                vlast_d[:],
                arena[:, 2 * NPH * SCOL:(2 * NPH + 2) * SCOL],
            ).then_inc(out_sem, 16)
            sync.wait_ge(act_sem, NB + 1)
            sync.dma_start(red_d[:, :], red_sb[:, :]).then_inc(out_sem, 16)
            sync.wait_ge(out_sem, 32)

        @block.scalar
        def _(scalar):
            # dependency-free dummy exp preloads the ACT exp table
            nc.scalar.activation(ex_sb[:, 0:1], emq_sb[:, 0:1], Exp)
            scalar.wait_ge(dve_sem, 1)  # bias memset
            for bi, (b0, b1) in enumerate(EXP_BLOCKS):
                lo, hi = b0 * SCOL, b1 * SCOL
                scalar.wait_ge(em_sem, 16 * (BLOCK_PART[bi] + 1))
                nc.scalar.activation(
                    ex_sb[:, lo:hi], emq_sb[:, lo:hi], Exp,
                    bias=bias_sb[:], scale=1.0 / QS,
                ).then_inc(act_sem, 1)
            # evict reduction rows psum -> sbuf; the final-phase reductions
            # are emitted before the last main MMs, so this hides in-stream
            scalar.wait_ge(pe_sem, last_red_count)
            nc.scalar.activation(red_sb[:], psr[:], Copy).then_inc(act_sem, 1)

        @block.vector
        def _(vector):
            nc.vector.memset(bias_sb[:], -CLVL).then_inc(dve_sem, 1)
            nc.vector.memset(arena[:, BL:SCOL], 1.0 / T).then_inc(dve_sem, 1)
            nc.vector.memset(arena[:, SCOL:2 * SCOL], 1.0 / T).then_inc(dve_sem, 1)
            vector.wait_ge(dma_in, 16)
            nc.vector.tensor_copy(arena[:, 0:BL], init_ap).then_inc(dve_sem, 1)
            prev_blk = -1
            for p in range(1, NPH + 1):
                for s in range(2):
                    hu = 2 * (p - 1) + s
                    if blk_of_hu[hu] != prev_blk:
                        vector.wait_ge(act_sem, 1 + blk_of_hu[hu])
                        prev_blk = blk_of_hu[hu]
                    q = hu % 4
                    col = (p - 1) * PCOL + s * SCOL
                    vector.wait_ge(pe_sem, mm_index[(p, s)])
                    nc.vector.tensor_tensor(
                        astate(s, p), psb[q][:], ex_sb[:, col:col + SCOL], mult,
                    ).then_inc(dve_sem, 1)

        @block.tensor
        def _(tensor):
            mm_ctr = 0

            def red_mm(s, p):
                nonlocal mm_ctr
                k = red_idx[(s, p)]
                nc.tensor.matmul(
                    psr[:], evstrip[:, 31 - k:63 - k], astate(s, p),
                    start=(k == 0), stop=(k == NRED - 1), skip_group_check=True,
                ).then_inc(pe_sem, 1)
                mm_ctr += 1

            # p-state preheat: dependency-free dummies keep PE continuously
            # busy until the real stream starts, reaching full clock early
            for _ in range(7):
                nc.tensor.matmul(psd[:], E_ap, arena[:, 0:SCOL],
                                 start=True, stop=True)
            tensor.wait_ge(dve_sem, 3)   # state-0 memsets done
            tensor.wait_ge(dma_in, 16)   # params (E + evstrip)
            for s in range(2):
                if (s, 0) in red_idx:    # only for sequences shorter than WIN
                    tensor.wait_ge(dve_sem, 4)   # chain-0 init copy
                    red_mm(s, 0)
            for p in range(1, NPH + 1):
                for s in range(2):
                    q = (2 * (p - 1) + s) % 4
                    if p >= 2:
                        # TT(p-1,s) done: covers rhs RAW + psum WAR (TT(p-2,s))
                        tensor.wait_ge(dve_sem, tt_done(p - 1, s))
                    # the last reduction goes before its co-dependent MM
                    if p == NPH and s == 1 and (s, p - 1) in red_idx:
                        red_mm(s, p - 1)
                    if p == 1 and s == 0:
                        # chain-0 state-0 columns read straight from the params
                        # view, so this MM is not gated on the DVE init copy
                        nc.tensor.matmul(psb[q][:, 0:BL], E_ap, init_ap,
                                         start=True, stop=True)
                        nc.tensor.matmul(
                            psb[q][:, BL:SCOL], E_ap, arena[:, BL:SCOL],
                            start=True, stop=True,
                        ).then_inc(pe_sem, 1)
                        mm_ctr += 1
                        assert mm_index[(p, s)] == mm_ctr
                        continue
                    nc.tensor.matmul(
                        psb[q][:], E_ap, astate(s, p - 1),
                        start=True, stop=True,
                    ).then_inc(pe_sem, 1)
                    mm_ctr += 1
                    assert mm_index[(p, s)] == mm_ctr
                    if (s, p - 1) in red_idx and p >= 2 and not (p == NPH and s == 1):
                        red_mm(s, p - 1)
            assert mm_ctr == n_mms, (mm_ctr, n_mms)

    return nc


def _host_prep(inputs, transitions, start_transitions, end_transitions):
    """Per-core input maps; emissions quantized+gathered into consumption order."""
    tindex = np.empty((NPH, G), dtype=np.int64)
    for p in range(1, NPH + 1):
        for g in range(G):
            tindex[p - 1, g] = min(_t_of(g, p), L - 1)

    q = np.clip(np.round(QS * inputs), -127, 127).astype(np.int8)  # [B, L, T]
    shared = np.zeros((T, 256), dtype=bf16)
    shared[:, 0:128] = np.exp(transitions.astype(np.float64)).astype(bf16)
    shared[:, 160 + 31] = np.exp(end_transitions.astype(np.float64)).astype(bf16)

    in_maps = []
    for i in range(NCORES):
        qc = q[i * BL:(i + 1) * BL]                       # [32, 512, 128]
        qT = np.ascontiguousarray(qc.transpose(2, 1, 0))  # [j, t, b]
        emq = np.ascontiguousarray(qT[:, tindex, :]).reshape(T, NPH * PCOL)
        core = inputs[i * BL:(i + 1) * BL]
        params = shared.copy()
        params[:, 128:160] = np.exp(
            start_transitions.astype(np.float64)[:, None]
            + core[:, 0, :].T.astype(np.float64)).astype(bf16)
        in_maps.append({"emq": emq, "params": params})
    return in_maps


def _host_finish(results, red_states, inputs, transitions, start_transitions,
                 end_transitions, tags, mask):
    red_idx = {sp: k for k, sp in enumerate(red_states)}
    maskf = mask.astype(np.float64)
    lengths = mask.astype(np.int64).sum(axis=1)
    expend = np.exp(end_transitions.astype(np.float64))

    total = 0.0
    for i in range(NCORES):
        red = np.asarray(results[i]["red"]).astype(np.float64)      # [32, 512]
        vlast = np.asarray(results[i]["vlast"]).astype(np.float64)  # [T, 2*SCOL]
        rlast = expend @ vlast  # expend-weighted sums of the final states

        def r(g, p):
            """expend-weighted sums for chain g state p: [BL] vector."""
            s = g // CPS
            c0 = (g % CPS) * BL
            if p == NPH:
                return rlast[s * SCOL + c0:s * SCOL + c0 + BL]
            return red[red_idx[(s, p)]][c0:c0 + BL]

        lvl = np.zeros((G, BL))
        for g in range(1, G):
            p_prev = WIN if g == 1 else NPH
            lvl[g] = (np.log(r(g - 1, p_prev)) + lvl[g - 1] + p_prev * CLVL
                      - (np.log(r(g, W)) + W * CLVL))

        bs = slice(i * BL, (i + 1) * BL)
        log_den = np.zeros(BL)
        for bb in range(BL):
            t = int(lengths[bs][bb]) - 1
            g, p = _endpoint_state(t)
            log_den[bb] = np.log(r(g, p)[bb]) + lvl[g, bb] + p * CLVL
        total += -log_den.sum()

    # numerator (gold-path score) — cheap gathers over [B, L]
    tg = tags.astype(np.int64)
    b_idx = np.arange(B)
    inp = inputs.astype(np.float64)
    score = start_transitions.astype(np.float64)[tg[:, 0]]
    trans_sc = transitions.astype(np.float64)[tg[:, :-1], tg[:, 1:]]
    emit = np.take_along_axis(inp, tg[:, :, None], axis=2)[..., 0]
    score = score + (trans_sc * maskf[:, 1:]).sum(axis=1)
    score = score + (emit[:, :-1] * maskf[:, :-1]).sum(axis=1)
    last_tags = tg[b_idx, lengths - 1]
    score = score + end_transitions.astype(np.float64)[last_tags]
    score = score + inp[:, -1][b_idx, last_tags] * maskf[:, -1]
    total += score.sum()
    return np.float32(total)


def _run(inputs, transitions, start_transitions, end_transitions, tags, mask,
         trace=False):
    from concourse.bass_utils import run_bass_kernel_spmd

    inputs = np.asarray(inputs, dtype=np.float32)
    transitions = np.asarray(transitions, dtype=np.float32)
    start_transitions = np.asarray(start_transitions, dtype=np.float32)
    end_transitions = np.asarray(end_transitions, dtype=np.float32)
    tags = np.asarray(tags)
    mask = np.asarray(mask)

    lengths = mask.astype(np.int64).sum(axis=1)
    red_states = _red_states(lengths)
    nc = _build_nc(red_states)
    in_maps = _host_prep(inputs, transitions, start_transitions, end_transitions)
    res = run_bass_kernel_spmd(nc, in_maps, list(range(NCORES)), trace=trace)
    out = _host_finish(res.results, red_states, inputs, transitions,
                       start_transitions, end_transitions, tags, mask)
    return out, res, red_states


def kernel(inputs, transitions, start_transitions, end_transitions, tags, mask):
    out, _, _ = _run(inputs, transitions, start_transitions, end_transitions,
                     tags, mask)
    return out
